# revision 78
# baseline (speedup 1.0000x reference)
"""Trainium2 Bass kernel for nn_JinaPairTraining (dense CE + late-interaction
maxsim CE + KL between the two softmax distributions).

Sharding: data-parallel over queries, mask-compacted on both sides.

q side: the 32 queries are binned 4-per-core balancing *valid* token counts;
each core packs its bins' valid q tokens densely into J 128-lane chunks
(lanes freely mix queries -- the one-hot sum-over-q matmul untangles them, so
masked q tokens are never computed). p side: each doc's valid prefix is
padded to a multiple of 64 tokens (min 128), docs are grouped by padded size
into PSUM chunks of <= 512 folded columns (one segmented reduce per
equal-size run), so masked pos work is mostly skipped.

Device-side structure (per core):
  * Q/P matmuls in fp8e4m3 (inputs scaled x8; ~6e-4 final rel err) with
    DoubleRow packing (contraction 128 = 64 partitions x 2) -- 2x PE rate.
  * p_mask folded on the host (masked tokens replaced by the doc's first
    valid token -- duplicates never change a max).
  * Pair-max folded into PE+ACT: max(s0,s1) = s1 + relu(s0-s1); PE computes
    Q@(P0-P1) and Q@P1 (DoubleRow), ACT applies relu, an identity matmul
    accumulates, DVE does one segmented reduce_max per chunk. The PSUM drain
    is split ~50/50 between ACT (diffs) and DVE (folded maxes), which is the
    binding resource; PSUM banks: 1 logits + 4 diff-tiles + 3 max-tiles.
  * 1/t_i, 1/tau and the fp8 scale correction are folded into the one-hot
    matmul operands; logits use a constant exp shift (cosine |z| <= 50).
  * The dense-CE side, the per-chunk-j logits accumulation, and their
    exp/diag/KL-cross sums run concurrently with the main loop against a
    dedicated PSUM bank; the device returns 6 row stats (exp-sums, diagonal
    logits, KL cross sums) and the host finishes with a few scalar logs.
  * Input DMAs are spread across the ACT/Pool/SP queues so their fixed
    overheads overlap; a tiny warm-up matmul at t~0 starts the PE ramp early.
"""

import os
import sys

import numpy as np

for _p in ("/opt/trn_rl_repo",):
    if _p not in sys.path and os.path.isdir(_p):
        sys.path.insert(0, _p)

import ml_dtypes

import concourse.bacc as bacc
import concourse.tile as tile
from concourse import mybir
from concourse.bass_utils import run_bass_kernel_spmd

B, T, D = 32, 256, 128
TAU = 0.02
ITAU = 1.0 / TAU
NCORES = 8
ROWS = B // NCORES          # 4 query rows per core
SCALE = 8.0                 # fp8 pre-scale; sims come out x64
SIM_SCALE = SCALE * SCALE
ZBIAS = -25.0               # safe exp shift: |z| <= 50 for cosine logits

F32 = mybir.dt.float32
BF16 = mybir.dt.bfloat16
FP8 = mybir.dt.float8e4
AX = mybir.AxisListType
ALU = mybir.AluOpType
ACT = mybir.ActivationFunctionType
PM = mybir.MatmulPerfMode
NPF8 = ml_dtypes.float8_e4m3

# smalls layout (f32 [128, 160]): qoh cols 0:4J, qsT 32:36, psT 36:68,
# diag2 (partitions 0:ROWS) 68:132
SM_QS = 32
SM_PS = 36
SM_DG = 68
SM_W = 160


def _to_dr(a):
    """[X, D] (row-major, D=128 contraction) -> DoubleRow layout [64, 2, X]."""
    return np.ascontiguousarray(a.T.reshape(2, 64, -1).transpose(1, 0, 2))


def _build_kernel(J, plan):
    """plan: tuple of chunks; each chunk is a tuple of (seg, count) runs
    over folded columns."""
    chunk_sizes = [sum(s * c for s, c in ch) for ch in plan]
    fcols = sum(chunk_sizes)

    nc = bacc.Bacc(None, target_bir_lowering=False, debug=False)

    p8_d = nc.dram_tensor("p8", [64, 2, 2, fcols], FP8, kind="ExternalInput")
    q8_d = nc.dram_tensor("q8", [64, 2, J * 128], FP8, kind="ExternalInput")
    ident_d = nc.dram_tensor("ident", [128, 128], BF16, kind="ExternalInput")
    smalls_d = nc.dram_tensor("smalls", [128, SM_W], F32, kind="ExternalInput")
    out_d = nc.dram_tensor("out", [ROWS, 6], F32, kind="ExternalOutput")

    with tile.TileContext(nc) as tc:
        with (
            tc.tile_pool(name="sb", bufs=1) as sb,
            tc.tile_pool(name="rp", bufs=3) as rp,
        ):
            p8 = sb.tile([64, 2, 2, fcols], FP8, name="p8")
            q8 = sb.tile([64, 2, J * 128], FP8, name="q8")
            ident = sb.tile([128, 128], BF16, name="ident")
            smalls = sb.tile([128, SM_W], F32, name="smalls")
            mx = sb.tile([128, J, B], F32, name="mx")
            warm_a = sb.tile([128, 2], BF16, name="warm_a")
            warm_f = sb.tile([1, 2], F32, name="warm_f")

            # PE ramp + ACT exp-table warm-up at t~0 (1 PSUM bank, freed
            # before the main pools open).
            nc.vector.memset(warm_a, 0.0)
            with tc.tile_pool(name="wps", bufs=1, space="PSUM") as wpsp:
                wps = wpsp.tile([1, 2], F32, name="wps")
                nc.tensor.matmul(
                    wps, warm_a[:, 0:1], warm_a, start=True, stop=True
                )
                nc.scalar.activation(warm_f, wps, ACT.Exp)
                nc.scalar.activation(warm_f, warm_f, ACT.Relu)

            # Input DMAs: p regions own the HWDGE path (region 0 gates the
            # first matmuls); q8/ident/smalls ride the Pool SWDGE queue in
            # need-order (smalls is tail-only data).
            nc.scalar.dma_start(out=q8, in_=q8_d[:, :, :])
            nc.gpsimd.dma_start(out=smalls, in_=smalls_d[:, :])
            nc.gpsimd.dma_start(out=ident, in_=ident_d[:, :])
            # p DMAs chunk-aligned, ~4 chunks (<=2048 folded cols) each
            chunk_off = [0]
            for cs in chunk_sizes:
                chunk_off.append(chunk_off[-1] + cs)
            dma_cuts = sorted(
                {chunk_off[0], chunk_off[1]}
                | {chunk_off[i] for i in range(1, len(chunk_sizes), 1)}
                | {fcols}
            )
            for a, b in zip(dma_cuts[:-1], dma_cuts[1:]):
                nc.sync.dma_start(
                    out=p8[:, :, :, a:b], in_=p8_d[:, :, :, a:b]
                )

            # doc-position offset of each chunk in sorted-doc order
            dpos_off = [0]
            for ch in plan:
                dpos_off.append(dpos_off[-1] + sum(c for _, c in ch))

            ez = sb.tile([ROWS, 2 * B], F32, name="ez")
            zbias = sb.tile([ROWS, 1], F32, name="zbias")
            nc.vector.memset(zbias, ZBIAS)
            out_sb = sb.tile([ROWS, 6], F32, name="out_sb")
            junk = sb.tile([ROWS, 2 * B], F32, name="junk")
            wk = sb.tile([ROWS, 2 * B], F32, name="wk")

            with (
                tc.tile_pool(name="zp", bufs=1, space="PSUM") as zp,
                tc.tile_pool(name="pd", bufs=3, space="PSUM") as pdp,
                tc.tile_pool(name="pm", bufs=4, space="PSUM") as pmp,
            ):
                # dense-logit side runs entirely during the prologue/main
                z = zp.tile([ROWS, 2 * B], F32, name="z")
                nc.tensor.matmul(
                    z[:, 0:B],
                    smalls[:, SM_QS : SM_QS + ROWS],
                    smalls[:, SM_PS : SM_PS + B],
                    start=True,
                    stop=True,
                )
                nc.scalar.activation(ez[:, 0:B], z[:, 0:B], ACT.Exp, bias=zbias)
                nc.vector.reduce_sum(out=out_sb[:, 0:1], in_=ez[:, 0:B], axis=AX.X)
                nc.vector.scalar_tensor_tensor(
                    out=junk[:, 0:B], in0=z[:, 0:B], scalar=1.0,
                    in1=smalls[0:ROWS, SM_DG : SM_DG + B],
                    op0=ALU.mult, op1=ALU.mult, accum_out=out_sb[:, 2:3],
                )
                nc.vector.scalar_tensor_tensor(
                    out=wk[:, 0:B], in0=z[:, 0:B], scalar=1.0, in1=ez[:, 0:B],
                    op0=ALU.mult, op1=ALU.mult, accum_out=out_sb[:, 4:5],
                )

                # chunk-outer, j-inner: the first p chunk alone feeds J
                # iterations, hiding the p-DMA pipeline behind compute
                for ci, runs in enumerate(plan):
                    size = chunk_sizes[ci]
                    base = chunk_off[ci]
                    last_chunk = ci == len(plan) - 1
                    # split the very first iteration into two sub-chunks so
                    # the DVE conveyor's first reduce starts ~300ns earlier
                    # (one-time overhead, ramp-only)
                    first_split = (
                        ci == 0
                        and len(runs) == 1
                        and runs[0][1] >= 2
                        and runs[0][0] * (runs[0][1] // 2) >= 128
                    )
                    for j in range(J):
                        if ci == 0 and j == 0 and first_split:
                            seg0, cnt0 = runs[0]
                            ca_ = cnt0 // 2
                            sub = (
                                (0, ((seg0, ca_),)),
                                (seg0 * ca_, ((seg0, cnt0 - ca_),)),
                            )
                        else:
                            sub = ((0, runs),)
                        qj = q8[:, :, j * 128 : (j + 1) * 128]
                        for sub_off, sub_runs in sub:
                            sub_size = sum(s * c for s, c in sub_runs)
                            sbase = base + sub_off
                            sdpos = dpos_off[ci] + sub_off // sub_runs[0][0]
                            ps_d = pdp.tile([128, 512], F32, name="ps_d")
                            nc.tensor.matmul(
                                ps_d[:, 0:sub_size],
                                qj,
                                p8[:, :, 1, sbase : sbase + sub_size],
                                start=True,
                                stop=True,
                                perf_mode=PM.DoubleRow,
                            )
                            relu = rp.tile([128, 512], BF16, name="relu")
                            nc.scalar.activation(
                                relu[:, 0:sub_size], ps_d[:, 0:sub_size], ACT.Relu
                            )
                            ps_m = pmp.tile([128, 512], F32, name="ps_m")
                            nc.tensor.matmul(
                                ps_m[:, 0:sub_size],
                                qj,
                                p8[:, :, 0, sbase : sbase + sub_size],
                                start=True,
                                stop=False,
                                perf_mode=PM.DoubleRow,
                            )
                            nc.tensor.matmul(
                                ps_m[:, 0:sub_size],
                                ident,
                                relu[:, 0:sub_size],
                                start=False,
                                stop=True,
                            )
                            dpos = sdpos
                            o = 0
                            for seg, cnt in sub_runs:
                                nc.vector.reduce_max(
                                    out=mx[:, j, dpos : dpos + cnt],
                                    in_=ps_m[:, o : o + seg * cnt].rearrange(
                                        "p (g s) -> p g s", s=seg
                                    ),
                                    axis=AX.X,
                                )
                                o += seg * cnt
                                dpos += cnt
                        if last_chunk:
                            nc.tensor.matmul(
                                z[:, B : 2 * B],
                                smalls[:, j * ROWS : (j + 1) * ROWS],
                                mx[:, j, :],
                                start=(j == 0),
                                stop=(j == J - 1),
                            )
                        continue_marker = True
                    continue
                    # (unreachable original body below)
                    for j in range(J):
                        qj = q8[:, :, j * 128 : (j + 1) * 128]
                        ps_d = pdp.tile([128, 512], F32, name="ps_d")
                        nc.tensor.matmul(
                            ps_d[:, 0:size],
                            qj,
                            p8[:, :, 1, base : base + size],
                            start=True,
                            stop=True,
                            perf_mode=PM.DoubleRow,
                        )
                        relu = rp.tile([128, 512], BF16, name="relu")
                        nc.scalar.activation(
                            relu[:, 0:size], ps_d[:, 0:size], ACT.Relu
                        )
                        ps_m = pmp.tile([128, 512], F32, name="ps_m")
                        nc.tensor.matmul(
                            ps_m[:, 0:size],
                            qj,
                            p8[:, :, 0, base : base + size],
                            start=True,
                            stop=False,
                            perf_mode=PM.DoubleRow,
                        )
                        nc.tensor.matmul(
                            ps_m[:, 0:size],
                            ident,
                            relu[:, 0:size],
                            start=False,
                            stop=True,
                        )
                        dpos = dpos_off[ci]
                        o = 0
                        for seg, cnt in runs:
                            nc.vector.reduce_max(
                                out=mx[:, j, dpos : dpos + cnt],
                                in_=ps_m[:, o : o + seg * cnt].rearrange(
                                    "p (g s) -> p g s", s=seg
                                ),
                                axis=AX.X,
                            )
                            o += seg * cnt
                            dpos += cnt
                        if last_chunk:
                            # all chunks have processed j -> mx[:, j] complete
                            nc.tensor.matmul(
                                z[:, B : 2 * B],
                                smalls[:, j * ROWS : (j + 1) * ROWS],
                                mx[:, j, :],
                                start=(j == 0),
                                stop=(j == J - 1),
                            )

                # late tail: exp-sum, diag logit, KL cross term
                nc.vector.scalar_tensor_tensor(
                    out=junk[:, B : 2 * B], in0=z[:, B : 2 * B], scalar=1.0,
                    in1=smalls[0:ROWS, SM_DG + B : SM_DG + 2 * B],
                    op0=ALU.mult, op1=ALU.mult, accum_out=out_sb[:, 3:4],
                )
                nc.vector.scalar_tensor_tensor(
                    out=wk[:, B : 2 * B], in0=z[:, B : 2 * B], scalar=1.0,
                    in1=ez[:, 0:B],
                    op0=ALU.mult, op1=ALU.mult, accum_out=out_sb[:, 5:6],
                )
                ez_l = sb.tile([ROWS, B], F32, name="ez_l")
                nc.scalar.activation(
                    ez_l, z[:, B : 2 * B], ACT.Exp, bias=zbias,
                    accum_out=out_sb[:, 1:2],
                )

                nc.sync.dma_start(out=out_d[:, :], in_=out_sb)

    nc.compile()
    return nc


_NC_CACHE = {}
_NC_LAST = None


def _get_nc(key=None):
    global _NC_LAST
    if key is None:
        assert _NC_LAST is not None
        return _NC_LAST
    if key not in _NC_CACHE:
        _NC_CACHE[key] = _build_kernel(*key)
    _NC_LAST = _NC_CACHE[key]
    return _NC_LAST


def _plan_p(pmask):
    """Doc-compaction plan: pad each doc's valid prefix to a multiple of 64
    tokens (>= 128), sort docs by padded size (desc), split each size class
    into single-seg chunks of <= 512 folded cols. Returns (doc order, per-doc
    folded seg, chunk plan); plan entries are (seg, count)."""
    p_len = np.maximum(pmask.sum(axis=1).astype(np.int64), 1)
    pad = np.clip(-(-p_len // 64) * 64, 128, T)
    order = np.argsort(-pad, kind="stable")
    segs = (pad[order] // 2).astype(int)
    chunks = []
    cur = []
    cur_cols = 0
    for s in segs:
        s = int(s)
        if cur_cols + s > 512:
            chunks.append(cur)
            cur = []
            cur_cols = 0
        cur.append(s)
        cur_cols += s
    if cur:
        chunks.append(cur)
    plan = []
    for ch in chunks:
        runs = []
        for s in ch:
            if runs and runs[-1][0] == s:
                runs[-1][1] += 1
            else:
                runs.append([s, 1])
        plan.append(tuple((s, c) for s, c in runs))
    return order, segs, tuple(plan)


def _plan_cores(qmask):
    """Bin the 32 queries 4-per-core, balancing total *valid* q tokens.
    Valid tokens from a core's queries are packed densely into 128-lane
    chunks (lanes may mix queries -- the one-hot sum matmul untangles)."""
    q_len = np.maximum(qmask.sum(axis=1).astype(np.int64), 1)
    order = np.argsort(-q_len, kind="stable")
    bins = [[] for _ in range(NCORES)]
    sums = [0] * NCORES
    for b in order:
        cand = min(
            (i for i in range(NCORES) if len(bins[i]) < ROWS),
            key=lambda i: (sums[i], len(bins[i])),
        )
        bins[cand].append(int(b))
        sums[cand] += int(q_len[b])
    # swap refinement: chunks/core is set by the fullest bin, so push the
    # max-bin down toward ceil(total/(128*NCORES)) via pairwise swaps
    target = -(-int(q_len.sum()) // (128 * NCORES)) * 128
    for _ in range(64):
        hi = int(np.argmax(sums))
        if sums[hi] <= target:
            break
        best = None
        for lo in range(NCORES):
            if lo == hi:
                continue
            for a in bins[hi]:
                for b in bins[lo]:
                    d = int(q_len[a] - q_len[b])
                    if d <= 0:
                        continue
                    new_hi = sums[hi] - d
                    new_lo = sums[lo] + d
                    if max(new_hi, new_lo) < sums[hi] and (
                        best is None or max(new_hi, new_lo) < best[0]
                    ):
                        best = (max(new_hi, new_lo), lo, a, b)
        if best is None:
            break
        _, lo, a, b = best
        bins[hi].remove(a)
        bins[lo].remove(b)
        bins[hi].append(b)
        bins[lo].append(a)
        sums[hi] += int(q_len[b] - q_len[a])
        sums[lo] += int(q_len[a] - q_len[b])
    J = max(-(-s // 128) for s in sums)
    return bins, q_len, J


def _prep_in_maps(query_single, pos_single, query_multi, pos_multi, q_mask, p_mask):
    qs = np.asarray(query_single, np.float32)
    ps = np.asarray(pos_single, np.float32)
    qm = np.asarray(query_multi, np.float32)
    pm = np.asarray(pos_multi, np.float32)
    qmask = np.asarray(q_mask).astype(bool)
    pmask = np.asarray(p_mask).astype(bool)

    # Doc compaction + pair fold: gather each doc's valid tokens, pad to
    # 2*seg with copies of the first valid token (duplicates never change a
    # max). For doc d, column block = [P1 = tokens seg:2seg | Pd = tokens
    # 0:seg - P1], in sorted-doc order. P in DoubleRow fp8 [64,2,{P1,Pd},:].
    p_order, p_segs, plan = _plan_p(pmask)
    fcols = int(p_segs.sum())
    p1 = np.empty((fcols, D), np.float32)
    pd = np.empty((fcols, D), np.float32)
    o = 0
    for d, seg in zip(p_order, p_segs):
        valid = pm[d, pmask[d]]
        blk = np.empty((2 * seg, D), np.float32)
        n = min(len(valid), 2 * seg)
        blk[:n] = valid[:n]
        blk[n:] = valid[0]
        hi = blk[seg : 2 * seg]
        lo = blk[0:seg]
        p1[o : o + seg] = hi
        pd[o : o + seg] = lo - hi
        o += seg
    p8 = np.stack(
        [_to_dr(p1 * SCALE), _to_dr(pd * SCALE)], axis=2
    ).astype(NPF8)
    p8 = np.ascontiguousarray(p8)
    p_pos = np.empty(B, np.int64)  # doc -> column position in sorted order
    p_pos[p_order] = np.arange(B)

    ident = np.ascontiguousarray(np.eye(128, dtype=ml_dtypes.bfloat16))
    t_i = np.maximum(qmask.sum(axis=1), 1).astype(np.float64)

    bins, q_len, J = _plan_cores(qmask)

    in_maps = []
    for i in range(NCORES):
        # densely pack the bin's valid q tokens into J 128-lane chunks
        toks = np.concatenate(
            [
                np.stack(
                    [np.full(q_len[b], row), np.nonzero(qmask[b])[0]], axis=1
                )
                for row, b in enumerate(bins[i])
            ]
        )  # [n_tok, (row, t)]
        assert len(toks) <= J * 128
        qcat = np.zeros((J * 128, D), np.float32)
        qcat[: len(toks)] = qm[np.array(bins[i])[toks[:, 0]], toks[:, 1], :]
        q8 = _to_dr(qcat * SCALE).astype(NPF8)
        smalls = np.zeros((128, SM_W), np.float32)
        for j in range(J):
            for lane, (row, _t) in enumerate(toks[j * 128 : (j + 1) * 128]):
                b = bins[i][row]
                smalls[lane, j * ROWS + row] = ITAU / (SIM_SCALE * t_i[b])
        smalls[:, SM_QS : SM_QS + ROWS] = qs[bins[i]].T * ITAU
        smalls[:, SM_PS : SM_PS + B] = ps[p_order].T
        for row, b in enumerate(bins[i]):
            smalls[row, SM_DG + p_pos[b]] = 1.0
            smalls[row, SM_DG + B + p_pos[b]] = 1.0
        in_maps.append(
            {"p8": p8, "q8": q8, "ident": ident, "smalls": smalls}
        )
    return in_maps, (J, plan)


def run(inputs: dict, trace: bool = False):
    """Run the spmd kernel; returns (loss tuple, BassKernelResults)."""
    in_maps, key = _prep_in_maps(**inputs)
    nc = _get_nc(key)
    res = run_bass_kernel_spmd(
        nc, in_maps, core_ids=list(range(NCORES)), trace=trace
    )
    rows = np.concatenate([r["out"] for r in res.results], axis=0).astype(
        np.float64
    )  # [32, 6] = den_d, den_l, ztgt_d, ztgt_l, skl_a, skl_b
    den_d, den_l, ztd, ztl, skl_a, skl_b = rows.T
    sl = (-ZBIAS) + np.log(den_d) - ztd
    ml = (-ZBIAS) + np.log(den_l) - ztl
    kl = (skl_a - skl_b) / den_d - np.log(den_d) + np.log(den_l)
    single = sl.mean()
    multi = ml.mean()
    klm = kl.mean()
    total = single + multi + klm
    out = (
        np.float32(total),
        np.float32(single),
        np.float32(multi),
        np.float32(klm),
    )
    return out, res


def kernel(query_single, pos_single, query_multi, pos_multi, q_mask, p_mask):
    out, _ = run(
        {
            "query_single": query_single,
            "pos_single": pos_single,
            "query_multi": query_multi,
            "pos_multi": pos_multi,
            "q_mask": q_mask,
            "p_mask": p_mask,
        }
    )
    return out


# revision 79
# speedup vs baseline: 1.0046x; 1.0046x over previous
"""Trainium2 Bass kernel for nn_JinaPairTraining (dense CE + late-interaction
maxsim CE + KL between the two softmax distributions).

Sharding: data-parallel over queries, mask-compacted on both sides.

q side: the 32 queries are binned 4-per-core balancing *valid* token counts;
each core packs its bins' valid q tokens densely into J 128-lane chunks
(lanes freely mix queries -- the one-hot sum-over-q matmul untangles them, so
masked q tokens are never computed). p side: each doc's valid prefix is
padded to a multiple of 64 tokens (min 128), docs are grouped by padded size
into PSUM chunks of <= 512 folded columns (one segmented reduce per
equal-size run), so masked pos work is mostly skipped.

Device-side structure (per core):
  * Q/P matmuls in fp8e4m3 (inputs scaled x8; ~6e-4 final rel err) with
    DoubleRow packing (contraction 128 = 64 partitions x 2) -- 2x PE rate.
  * p_mask folded on the host (masked tokens replaced by the doc's first
    valid token -- duplicates never change a max).
  * Pair-max folded into PE+ACT: max(s0,s1) = s1 + relu(s0-s1); PE computes
    Q@(P0-P1) and Q@P1 (DoubleRow), ACT applies relu, an identity matmul
    accumulates, DVE does one segmented reduce_max per chunk. The PSUM drain
    is split ~50/50 between ACT (diffs) and DVE (folded maxes), which is the
    binding resource; PSUM banks: 1 logits + 4 diff-tiles + 3 max-tiles.
  * 1/t_i, 1/tau and the fp8 scale correction are folded into the one-hot
    matmul operands; logits use a constant exp shift (cosine |z| <= 50).
  * The dense-CE side, the per-chunk-j logits accumulation, and their
    exp/diag/KL-cross sums run concurrently with the main loop against a
    dedicated PSUM bank; the device returns 6 row stats (exp-sums, diagonal
    logits, KL cross sums) and the host finishes with a few scalar logs.
  * Input DMAs are spread across the ACT/Pool/SP queues so their fixed
    overheads overlap; a tiny warm-up matmul at t~0 starts the PE ramp early.
"""

import os
import sys

import numpy as np

for _p in ("/opt/trn_rl_repo",):
    if _p not in sys.path and os.path.isdir(_p):
        sys.path.insert(0, _p)

import ml_dtypes

import concourse.bacc as bacc
import concourse.tile as tile
from concourse import mybir
from concourse.bass_utils import run_bass_kernel_spmd

B, T, D = 32, 256, 128
TAU = 0.02
ITAU = 1.0 / TAU
NCORES = 8
ROWS = B // NCORES          # 4 query rows per core
SCALE = 8.0                 # fp8 pre-scale; sims come out x64
SIM_SCALE = SCALE * SCALE
ZBIAS = -25.0               # safe exp shift: |z| <= 50 for cosine logits

F32 = mybir.dt.float32
BF16 = mybir.dt.bfloat16
FP8 = mybir.dt.float8e4
AX = mybir.AxisListType
ALU = mybir.AluOpType
ACT = mybir.ActivationFunctionType
PM = mybir.MatmulPerfMode
NPF8 = ml_dtypes.float8_e4m3

# smalls layout (f32 [128, 160]): qoh cols 0:4J, qsT 32:36, psT 36:68,
# diag2 (partitions 0:ROWS) 68:132
SM_QS = 32
SM_PS = 36
SM_DG = 68
SM_W = 160


def _to_dr(a):
    """[X, D] (row-major, D=128 contraction) -> DoubleRow layout [64, 2, X]."""
    return np.ascontiguousarray(a.T.reshape(2, 64, -1).transpose(1, 0, 2))


def _build_kernel(J, plan):
    """plan: tuple of chunks; each chunk is a tuple of (seg, count) runs
    over folded columns."""
    chunk_sizes = [sum(s * c for s, c in ch) for ch in plan]
    fcols = sum(chunk_sizes)

    nc = bacc.Bacc(None, target_bir_lowering=False, debug=False)

    p8_d = nc.dram_tensor("p8", [64, 2, 2, fcols], FP8, kind="ExternalInput")
    q8_d = nc.dram_tensor("q8", [64, 2, J * 128], FP8, kind="ExternalInput")
    ident_d = nc.dram_tensor("ident", [128, 128], BF16, kind="ExternalInput")
    smalls_d = nc.dram_tensor("smalls", [128, SM_W], F32, kind="ExternalInput")
    out_d = nc.dram_tensor("out", [ROWS, 6], F32, kind="ExternalOutput")

    with tile.TileContext(nc) as tc:
        with (
            tc.tile_pool(name="sb", bufs=1) as sb,
            tc.tile_pool(name="rp", bufs=3) as rp,
        ):
            p8 = sb.tile([64, 2, 2, fcols], FP8, name="p8")
            q8 = sb.tile([64, 2, J * 128], FP8, name="q8")
            ident = sb.tile([128, 128], BF16, name="ident")
            smalls = sb.tile([128, SM_W], F32, name="smalls")
            mx = sb.tile([128, J, B], F32, name="mx")
            warm_a = sb.tile([128, 2], BF16, name="warm_a")
            warm_f = sb.tile([1, 2], F32, name="warm_f")

            # PE ramp + ACT exp-table warm-up at t~0 (1 PSUM bank, freed
            # before the main pools open).
            nc.vector.memset(warm_a, 0.0)
            with tc.tile_pool(name="wps", bufs=1, space="PSUM") as wpsp:
                wps = wpsp.tile([1, 2], F32, name="wps")
                nc.tensor.matmul(
                    wps, warm_a[:, 0:1], warm_a, start=True, stop=True
                )
                nc.scalar.activation(warm_f, wps, ACT.Exp)
                nc.scalar.activation(warm_f, warm_f, ACT.Relu)

            # Input DMAs: p regions own the HWDGE path (region 0 gates the
            # first matmuls); q8/ident/smalls ride the Pool SWDGE queue in
            # need-order (smalls is tail-only data).
            nc.scalar.dma_start(out=q8, in_=q8_d[:, :, :])
            nc.gpsimd.dma_start(out=smalls, in_=smalls_d[:, :])
            nc.gpsimd.dma_start(out=ident, in_=ident_d[:, :])
            # p DMAs chunk-aligned, ~4 chunks (<=2048 folded cols) each
            chunk_off = [0]
            for cs in chunk_sizes:
                chunk_off.append(chunk_off[-1] + cs)
            dma_cuts = sorted(
                {chunk_off[0], chunk_off[1]}
                | {chunk_off[i] for i in range(1, len(chunk_sizes), 1)}
                | {fcols}
            )
            for a, b in zip(dma_cuts[:-1], dma_cuts[1:]):
                nc.sync.dma_start(
                    out=p8[:, :, :, a:b], in_=p8_d[:, :, :, a:b]
                )

            # doc-position offset of each chunk in sorted-doc order
            dpos_off = [0]
            for ch in plan:
                dpos_off.append(dpos_off[-1] + sum(c for _, c in ch))

            ez = sb.tile([ROWS, 2 * B], F32, name="ez")
            zbias = sb.tile([ROWS, 1], F32, name="zbias")
            nc.vector.memset(zbias, ZBIAS)
            out_sb = sb.tile([ROWS, 6], F32, name="out_sb")
            junk = sb.tile([ROWS, 2 * B], F32, name="junk")
            wk = sb.tile([ROWS, 2 * B], F32, name="wk")

            with (
                tc.tile_pool(name="zp", bufs=1, space="PSUM") as zp,
                tc.tile_pool(name="pd", bufs=3, space="PSUM") as pdp,
                tc.tile_pool(name="pm", bufs=4, space="PSUM") as pmp,
            ):
                # dense-logit side runs entirely during the prologue/main
                z = zp.tile([ROWS, 2 * B], F32, name="z")
                nc.tensor.matmul(
                    z[:, 0:B],
                    smalls[:, SM_QS : SM_QS + ROWS],
                    smalls[:, SM_PS : SM_PS + B],
                    start=True,
                    stop=True,
                )
                nc.scalar.activation(ez[:, 0:B], z[:, 0:B], ACT.Exp, bias=zbias)
                nc.vector.reduce_sum(out=out_sb[:, 0:1], in_=ez[:, 0:B], axis=AX.X)
                nc.vector.scalar_tensor_tensor(
                    out=junk[:, 0:B], in0=z[:, 0:B], scalar=1.0,
                    in1=smalls[0:ROWS, SM_DG : SM_DG + B],
                    op0=ALU.mult, op1=ALU.mult, accum_out=out_sb[:, 2:3],
                )
                nc.vector.scalar_tensor_tensor(
                    out=wk[:, 0:B], in0=z[:, 0:B], scalar=1.0, in1=ez[:, 0:B],
                    op0=ALU.mult, op1=ALU.mult, accum_out=out_sb[:, 4:5],
                )

                # chunk-outer, j-inner: the first p chunk alone feeds J
                # iterations, hiding the p-DMA pipeline behind compute
                for ci, runs in enumerate(plan):
                    size = chunk_sizes[ci]
                    base = chunk_off[ci]
                    last_chunk = ci == len(plan) - 1
                    for j in range(J):
                        qj = q8[:, :, j * 128 : (j + 1) * 128]
                        ps_d = pdp.tile([128, 512], F32, name="ps_d")
                        nc.tensor.matmul(
                            ps_d[:, 0:size],
                            qj,
                            p8[:, :, 1, base : base + size],
                            start=True,
                            stop=True,
                            perf_mode=PM.DoubleRow,
                        )
                        relu = rp.tile([128, 512], BF16, name="relu")
                        nc.scalar.activation(
                            relu[:, 0:size], ps_d[:, 0:size], ACT.Relu
                        )
                        ps_m = pmp.tile([128, 512], F32, name="ps_m")
                        nc.tensor.matmul(
                            ps_m[:, 0:size],
                            qj,
                            p8[:, :, 0, base : base + size],
                            start=True,
                            stop=False,
                            perf_mode=PM.DoubleRow,
                        )
                        nc.tensor.matmul(
                            ps_m[:, 0:size],
                            ident,
                            relu[:, 0:size],
                            start=False,
                            stop=True,
                        )
                        dpos = dpos_off[ci]
                        o = 0
                        for seg, cnt in runs:
                            nc.vector.reduce_max(
                                out=mx[:, j, dpos : dpos + cnt],
                                in_=ps_m[:, o : o + seg * cnt].rearrange(
                                    "p (g s) -> p g s", s=seg
                                ),
                                axis=AX.X,
                            )
                            o += seg * cnt
                            dpos += cnt
                        if last_chunk:
                            # all chunks have processed j -> mx[:, j] complete
                            nc.tensor.matmul(
                                z[:, B : 2 * B],
                                smalls[:, j * ROWS : (j + 1) * ROWS],
                                mx[:, j, :],
                                start=(j == 0),
                                stop=(j == J - 1),
                            )

                # late tail: exp-sum, diag logit, KL cross term
                nc.vector.scalar_tensor_tensor(
                    out=junk[:, B : 2 * B], in0=z[:, B : 2 * B], scalar=1.0,
                    in1=smalls[0:ROWS, SM_DG + B : SM_DG + 2 * B],
                    op0=ALU.mult, op1=ALU.mult, accum_out=out_sb[:, 3:4],
                )
                nc.vector.scalar_tensor_tensor(
                    out=wk[:, B : 2 * B], in0=z[:, B : 2 * B], scalar=1.0,
                    in1=ez[:, 0:B],
                    op0=ALU.mult, op1=ALU.mult, accum_out=out_sb[:, 5:6],
                )
                ez_l = sb.tile([ROWS, B], F32, name="ez_l")
                nc.scalar.activation(
                    ez_l, z[:, B : 2 * B], ACT.Exp, bias=zbias,
                    accum_out=out_sb[:, 1:2],
                )

                nc.sync.dma_start(out=out_d[:, :], in_=out_sb)

    nc.compile()
    return nc


_NC_CACHE = {}
_NC_LAST = None


def _get_nc(key=None):
    global _NC_LAST
    if key is None:
        assert _NC_LAST is not None
        return _NC_LAST
    if key not in _NC_CACHE:
        _NC_CACHE[key] = _build_kernel(*key)
    _NC_LAST = _NC_CACHE[key]
    return _NC_LAST


def _plan_p(pmask):
    """Doc-compaction plan: pad each doc's valid prefix to a multiple of 64
    tokens (>= 128), sort docs by padded size (desc), split each size class
    into single-seg chunks of <= 512 folded cols. Returns (doc order, per-doc
    folded seg, chunk plan); plan entries are (seg, count)."""
    p_len = np.maximum(pmask.sum(axis=1).astype(np.int64), 1)
    pad = np.clip(-(-p_len // 64) * 64, 128, T)
    order = np.argsort(-pad, kind="stable")
    segs = (pad[order] // 2).astype(int)
    chunks = []
    cur = []
    cur_cols = 0
    for s in segs:
        s = int(s)
        if cur_cols + s > 512:
            chunks.append(cur)
            cur = []
            cur_cols = 0
        cur.append(s)
        cur_cols += s
    if cur:
        chunks.append(cur)
    plan = []
    for ch in chunks:
        runs = []
        for s in ch:
            if runs and runs[-1][0] == s:
                runs[-1][1] += 1
            else:
                runs.append([s, 1])
        plan.append(tuple((s, c) for s, c in runs))
    return order, segs, tuple(plan)


def _plan_cores(qmask):
    """Bin the 32 queries 4-per-core, balancing total *valid* q tokens.
    Valid tokens from a core's queries are packed densely into 128-lane
    chunks (lanes may mix queries -- the one-hot sum matmul untangles)."""
    q_len = np.maximum(qmask.sum(axis=1).astype(np.int64), 1)
    order = np.argsort(-q_len, kind="stable")
    bins = [[] for _ in range(NCORES)]
    sums = [0] * NCORES
    for b in order:
        cand = min(
            (i for i in range(NCORES) if len(bins[i]) < ROWS),
            key=lambda i: (sums[i], len(bins[i])),
        )
        bins[cand].append(int(b))
        sums[cand] += int(q_len[b])
    # swap refinement: chunks/core is set by the fullest bin, so push the
    # max-bin down toward ceil(total/(128*NCORES)) via pairwise swaps
    target = -(-int(q_len.sum()) // (128 * NCORES)) * 128
    for _ in range(64):
        hi = int(np.argmax(sums))
        if sums[hi] <= target:
            break
        best = None
        for lo in range(NCORES):
            if lo == hi:
                continue
            for a in bins[hi]:
                for b in bins[lo]:
                    d = int(q_len[a] - q_len[b])
                    if d <= 0:
                        continue
                    new_hi = sums[hi] - d
                    new_lo = sums[lo] + d
                    if max(new_hi, new_lo) < sums[hi] and (
                        best is None or max(new_hi, new_lo) < best[0]
                    ):
                        best = (max(new_hi, new_lo), lo, a, b)
        if best is None:
            break
        _, lo, a, b = best
        bins[hi].remove(a)
        bins[lo].remove(b)
        bins[hi].append(b)
        bins[lo].append(a)
        sums[hi] += int(q_len[b] - q_len[a])
        sums[lo] += int(q_len[a] - q_len[b])
    J = max(-(-s // 128) for s in sums)
    return bins, q_len, J


def _prep_in_maps(query_single, pos_single, query_multi, pos_multi, q_mask, p_mask):
    qs = np.asarray(query_single, np.float32)
    ps = np.asarray(pos_single, np.float32)
    qm = np.asarray(query_multi, np.float32)
    pm = np.asarray(pos_multi, np.float32)
    qmask = np.asarray(q_mask).astype(bool)
    pmask = np.asarray(p_mask).astype(bool)

    # Doc compaction + pair fold: gather each doc's valid tokens, pad to
    # 2*seg with copies of the first valid token (duplicates never change a
    # max). For doc d, column block = [P1 = tokens seg:2seg | Pd = tokens
    # 0:seg - P1], in sorted-doc order. P in DoubleRow fp8 [64,2,{P1,Pd},:].
    p_order, p_segs, plan = _plan_p(pmask)
    fcols = int(p_segs.sum())
    p1 = np.empty((fcols, D), np.float32)
    pd = np.empty((fcols, D), np.float32)
    o = 0
    for d, seg in zip(p_order, p_segs):
        valid = pm[d, pmask[d]]
        blk = np.empty((2 * seg, D), np.float32)
        n = min(len(valid), 2 * seg)
        blk[:n] = valid[:n]
        blk[n:] = valid[0]
        hi = blk[seg : 2 * seg]
        lo = blk[0:seg]
        p1[o : o + seg] = hi
        pd[o : o + seg] = lo - hi
        o += seg
    p8 = np.stack(
        [_to_dr(p1 * SCALE), _to_dr(pd * SCALE)], axis=2
    ).astype(NPF8)
    p8 = np.ascontiguousarray(p8)
    p_pos = np.empty(B, np.int64)  # doc -> column position in sorted order
    p_pos[p_order] = np.arange(B)

    ident = np.ascontiguousarray(np.eye(128, dtype=ml_dtypes.bfloat16))
    t_i = np.maximum(qmask.sum(axis=1), 1).astype(np.float64)

    bins, q_len, J = _plan_cores(qmask)

    in_maps = []
    for i in range(NCORES):
        # densely pack the bin's valid q tokens into J 128-lane chunks
        toks = np.concatenate(
            [
                np.stack(
                    [np.full(q_len[b], row), np.nonzero(qmask[b])[0]], axis=1
                )
                for row, b in enumerate(bins[i])
            ]
        )  # [n_tok, (row, t)]
        assert len(toks) <= J * 128
        qcat = np.zeros((J * 128, D), np.float32)
        qcat[: len(toks)] = qm[np.array(bins[i])[toks[:, 0]], toks[:, 1], :]
        q8 = _to_dr(qcat * SCALE).astype(NPF8)
        smalls = np.zeros((128, SM_W), np.float32)
        for j in range(J):
            for lane, (row, _t) in enumerate(toks[j * 128 : (j + 1) * 128]):
                b = bins[i][row]
                smalls[lane, j * ROWS + row] = ITAU / (SIM_SCALE * t_i[b])
        smalls[:, SM_QS : SM_QS + ROWS] = qs[bins[i]].T * ITAU
        smalls[:, SM_PS : SM_PS + B] = ps[p_order].T
        for row, b in enumerate(bins[i]):
            smalls[row, SM_DG + p_pos[b]] = 1.0
            smalls[row, SM_DG + B + p_pos[b]] = 1.0
        in_maps.append(
            {"p8": p8, "q8": q8, "ident": ident, "smalls": smalls}
        )
    return in_maps, (J, plan)


def run(inputs: dict, trace: bool = False):
    """Run the spmd kernel; returns (loss tuple, BassKernelResults)."""
    in_maps, key = _prep_in_maps(**inputs)
    nc = _get_nc(key)
    res = run_bass_kernel_spmd(
        nc, in_maps, core_ids=list(range(NCORES)), trace=trace
    )
    rows = np.concatenate([r["out"] for r in res.results], axis=0).astype(
        np.float64
    )  # [32, 6] = den_d, den_l, ztgt_d, ztgt_l, skl_a, skl_b
    den_d, den_l, ztd, ztl, skl_a, skl_b = rows.T
    sl = (-ZBIAS) + np.log(den_d) - ztd
    ml = (-ZBIAS) + np.log(den_l) - ztl
    kl = (skl_a - skl_b) / den_d - np.log(den_d) + np.log(den_l)
    single = sl.mean()
    multi = ml.mean()
    klm = kl.mean()
    total = single + multi + klm
    out = (
        np.float32(total),
        np.float32(single),
        np.float32(multi),
        np.float32(klm),
    )
    return out, res


def kernel(query_single, pos_single, query_multi, pos_multi, q_mask, p_mask):
    out, _ = run(
        {
            "query_single": query_single,
            "pos_single": pos_single,
            "query_multi": query_multi,
            "pos_multi": pos_multi,
            "q_mask": q_mask,
            "p_mask": p_mask,
        }
    )
    return out
